# revision 7
# baseline (speedup 1.0000x reference)
"""Batched GAT layer (B=8, N=2048, Fin=256, Fout=128) on 8 Trainium2 NeuronCores.

Strategy: data-parallel over batch B — one batch element per core. Inside
each core a column-block formulation keeps the softmax contraction (over
neighbors j) on the PSUM accumulation path of the tensor engine:

  h      = x @ W.T + b                      (PE, fp32)
  e[j,i] = leakyrelu(s1[i] + s2[j])         s1 = h a1, s2 = h a2
  p      = exp(e + maskbias)                maskbias = 0 / -240 (fp8 from host)
  out    = elu((p.T scaled) ... )           h'T[o,i] = sum_j h[j,o] p[j,i] / S[i]

Host-side work is layout only: transposes, dtype packing of adj into an
additive fp8 mask, and the final un-transpose of the per-core outputs.
"""
import numpy as np
import ml_dtypes

B, N, FIN, FOUT = 8, 2048, 256, 128
P = 128
NT = N // P          # 16 j-tiles
NC4 = N // 512       # 4 psum chunks
ALPHA = 0.4
MASK_NEG = -240.0

# j-tiles whose leakyrelu runs on the vector engine instead of ACT (load
# balance knob), and j-tiles whose mask-add runs on gpsimd instead of DVE.
DVE_LEAKY_TILES = frozenset({2, 5, 8, 11, 14})
GPS_EM_TILES = frozenset()

_cache = {}


def _build():
    import concourse.mybir as mybir
    import concourse.tile as tile
    from concourse import bacc
    from concourse.masks import make_identity

    F32 = mybir.dt.float32
    F32R = mybir.dt.float32r
    FP8 = mybir.dt.float8e4
    AF = mybir.ActivationFunctionType
    ALU = mybir.AluOpType

    nc = bacc.Bacc("TRN2", target_bir_lowering=False, debug=False)

    xT_d = nc.dram_tensor("xT", [FIN, N], F32, kind="ExternalInput").ap()
    adjm_d = nc.dram_tensor("adjm", [N, N], FP8, kind="ExternalInput").ap()
    wt_d = nc.dram_tensor("wt", [FIN, FOUT], F32, kind="ExternalInput").ap()
    bcol_d = nc.dram_tensor("bcol", [FOUT, 1], F32, kind="ExternalInput").ap()
    a12_d = nc.dram_tensor("a12", [FOUT, 2], F32, kind="ExternalInput").ap()
    out_d = nc.dram_tensor("outT", [FOUT, N], F32, kind="ExternalOutput").ap()

    from contextlib import ExitStack
    with tile.TileContext(nc) as tc:
        with tc.tile_pool(name="const", bufs=1) as cpool, \
             tc.tile_pool(name="work", bufs=3) as wpool, \
             tc.tile_pool(name="adj", bufs=4) as apool:
            prep_ctx = ExitStack()
            pst = prep_ctx.enter_context(tc.tile_pool(name="pst", bufs=2, space="PSUM"))

            # ---- load constants / inputs ----
            xt0 = cpool.tile([P, N], F32, tag="xt0")
            xt1 = cpool.tile([P, N], F32, tag="xt1")
            nc.sync.dma_start(xt0[:], xT_d[0:P, :])
            nc.sync.dma_start(xt1[:], xT_d[P:FIN, :])
            wt0 = cpool.tile([P, FOUT], F32, tag="wt0")
            wt1 = cpool.tile([P, FOUT], F32, tag="wt1")
            nc.sync.dma_start(wt0[:], wt_d[0:P, :])
            nc.sync.dma_start(wt1[:], wt_d[P:FIN, :])
            bcol = cpool.tile([FOUT, 1], F32, tag="bcol")
            nc.sync.dma_start(bcol[:], bcol_d)
            a12 = cpool.tile([FOUT, 2], F32, tag="a12")
            nc.sync.dma_start(a12[:], a12_d)

            ident = cpool.tile([P, P], F32, tag="ident")
            make_identity(nc, ident[:])
            ones_col_f = cpool.tile([P, 1], F32, tag="ones_col_f")
            nc.gpsimd.memset(ones_col_f[:], 1.0)
            ones_col = cpool.tile([P, 1], F32R, tag="ones_col")
            nc.vector.tensor_copy(ones_col[:], ones_col_f[:])
            ones_row = cpool.tile([1, P], F32, tag="ones_row")
            nc.gpsimd.memset(ones_row[:], 1.0)

            # ---- hT[o, n] = W x + b  (fp32 matmuls, bias fused in ACT copy) ----
            hT = cpool.tile([FOUT, N], F32, tag="hT")
            for c in range(NC4):
                hps = pst.tile([FOUT, 512], F32, tag="tmp")
                sl = slice(c * 512, (c + 1) * 512)
                nc.tensor.matmul(hps[:], wt0[:], xt0[:, sl], start=True, stop=False)
                nc.tensor.matmul(hps[:], wt1[:], xt1[:, sl], start=False, stop=True)
                nc.scalar.activation(hT[:, sl], hps[:], AF.Identity, bias=bcol[:])

            # ---- h_nat[t] = hT[:, t].T via PE transpose (f32r for the big matmuls) ----
            h_nat = []
            for t in range(NT):
                tps = pst.tile([P, P], F32, tag="tmp")
                nc.tensor.transpose(tps[:], hT[:, t * P:(t + 1) * P], ident[:])
                hn = cpool.tile([P, P], F32R, tag=f"h_nat{t}")
                nc.vector.tensor_copy(hn[:], tps[:])
                h_nat.append(hn)

            # ---- s12[2, n] = [a1 a2].T @ hT ----
            s12 = cpool.tile([2, N], F32, tag="s12")
            for c in range(NC4):
                sps = pst.tile([2, 512], F32, tag="tmp")
                sl = slice(c * 512, (c + 1) * 512)
                nc.tensor.matmul(sps[:], a12[:], hT[:, sl], start=True, stop=True)
                nc.vector.tensor_copy(s12[:, sl], sps[:])

            # s2 as per-partition columns: s2_cols[p, t] = s2[t*128 + p]
            s2_cols = cpool.tile([P, NT], F32, tag="s2_cols")
            for t in range(NT):
                nc.sync.dma_start(s2_cols[:, t:t + 1], s12[1:2, t * P:(t + 1) * P])

            # ---- s1 broadcast to all partitions: s1b[p, i] = s1[i] ----
            s1b = cpool.tile([P, N], F32, tag="s1b")
            for c in range(NC4):
                bps = pst.tile([P, 512], F32, tag="tmp")
                sl = slice(c * 512, (c + 1) * 512)
                nc.tensor.matmul(bps[:], ones_row[:], s12[0:1, sl], start=True, stop=True)
                nc.scalar.activation(s1b[:, sl], bps[:], AF.Identity)

            # ---- psum accumulators for h'T and S ----
            prep_ctx.close()
            acc_ctx = ExitStack()
            psacc = acc_ctx.enter_context(tc.tile_pool(name="psacc", bufs=1, space="PSUM"))
            sv_ctx = ExitStack()
            pssv = sv_ctx.enter_context(tc.tile_pool(name="pssv", bufs=1, space="PSUM"))
            acc = [psacc.tile([FOUT, 512], F32, tag=f"acc{c}", name=f"acc{c}") for c in range(NC4)]
            svec = [pssv.tile([1, 512], F32, tag=f"svec{c}", name=f"svec{c}") for c in range(NC4)]

            # ---- main j-loop ----
            for t in range(NT):
                adjm_t = apool.tile([P, N], FP8, tag="adjm")
                nc.sync.dma_start(adjm_t[:], adjm_d[t * P:(t + 1) * P, :])

                s2c = s2_cols[:, t:t + 1]
                if t in GPS_EM_TILES:
                    # leaky on ACT straight from s1b (bias=s2), mask-add on gpsimd
                    lp = wpool.tile([P, N], F32, tag="em")
                    nc.scalar.activation(lp[:], s1b[:], AF.Prelu, bias=s2c,
                                         scale=1.0, alpha=ALPHA)
                    l_t = wpool.tile([P, N], F32, tag="lt")
                    nc.gpsimd.tensor_tensor(l_t[:], lp[:], adjm_t[:], ALU.add)
                elif t in DVE_LEAKY_TILES:
                    em = wpool.tile([P, N], F32, tag="em")
                    nc.vector.scalar_tensor_tensor(em[:], in0=s1b[:], scalar=s2c,
                                                   in1=adjm_t[:], op0=ALU.add, op1=ALU.add)
                    l_t = wpool.tile([P, N], F32, tag="lt")
                    nc.vector.scalar_tensor_tensor(l_t[:], in0=em[:], scalar=ALPHA,
                                                   in1=em[:], op0=ALU.mult, op1=ALU.max)
                else:
                    em = wpool.tile([P, N], F32, tag="em")
                    nc.vector.scalar_tensor_tensor(em[:], in0=s1b[:], scalar=s2c,
                                                   in1=adjm_t[:], op0=ALU.add, op1=ALU.add)
                    l_t = wpool.tile([P, N], F32, tag="lt")
                    nc.scalar.activation(l_t[:], em[:], AF.Prelu, bias=0.0,
                                         scale=1.0, alpha=ALPHA)
                p_t = wpool.tile([P, N], F32R, tag="pt")
                nc.scalar.activation(p_t[:], l_t[:], AF.Exp)

                first, last = (t == 0), (t == NT - 1)
                for c in range(NC4):
                    sl = slice(c * 512, (c + 1) * 512)
                    nc.tensor.matmul(acc[c][:], h_nat[t][:], p_t[:, sl],
                                     start=first, stop=last)
                for c in range(NC4):
                    sl = slice(c * 512, (c + 1) * 512)
                    nc.tensor.matmul(svec[c][:], ones_col[:], p_t[:, sl],
                                     start=first, stop=last)

            # ---- tail: normalize + elu ----
            s_row = cpool.tile([1, N], F32, tag="s_row")
            for c in range(NC4):
                nc.vector.tensor_copy(s_row[:, c * 512:(c + 1) * 512], svec[c][:])
            sv_ctx.close()
            # column-shuffle so reciprocal runs wide: sv_cols[p, c*4+t] = S[c*512 + p*4 + t]
            sv_cols = cpool.tile([P, 4 * NC4], F32, tag="sv_cols")
            for c in range(NC4):
                nc.sync.dma_start(sv_cols[:, c * 4:(c + 1) * 4], s_row[0:1, c * 512:(c + 1) * 512])
            rs_cols = cpool.tile([P, 4 * NC4], F32, tag="rs_cols")
            nc.vector.reciprocal(rs_cols[:], sv_cols[:])
            # un-shuffle with the inverse DMA mapping
            rs_row = cpool.tile([1, N], F32, tag="rs_row")
            for c in range(NC4):
                nc.sync.dma_start(rs_row[0:1, c * 512:(c + 1) * 512], rs_cols[:, c * 4:(c + 1) * 4])

            tail_ctx = ExitStack()
            pstail = tail_ctx.enter_context(tc.tile_pool(name="pstail", bufs=2, space="PSUM"))
            rb = cpool.tile([P, N], F32, tag="rb")
            for c in range(NC4):
                rps = pstail.tile([P, 512], F32, tag="rps")
                sl = slice(c * 512, (c + 1) * 512)
                nc.tensor.matmul(rps[:], ones_row[:], rs_row[0:1, sl], start=True, stop=True)
                nc.scalar.activation(rb[:, sl], rps[:], AF.Identity)

            hn_sb = cpool.tile([FOUT, N], F32, tag="hn_sb")
            for c in range(NC4):
                sl = slice(c * 512, (c + 1) * 512)
                nc.vector.tensor_tensor(hn_sb[:, sl], acc[c][:], rb[:, sl], ALU.mult)

            m0 = cpool.tile([FOUT, N], F32, tag="m0")
            nc.vector.tensor_scalar(m0[:], hn_sb[:], 0.0, None, op0=ALU.min)
            ex = cpool.tile([FOUT, N], F32, tag="ex")
            nc.scalar.activation(ex[:], m0[:], AF.Exp)
            outT = cpool.tile([FOUT, N], F32, tag="outT")
            nc.vector.scalar_tensor_tensor(outT[:], in0=ex[:], scalar=1.0,
                                           in1=hn_sb[:], op0=ALU.subtract, op1=ALU.max)
            nc.sync.dma_start(out_d, outT[:])
            tail_ctx.close()
            acc_ctx.close()

    nc.compile()
    return nc


def kernel(input, adj, W, b, a):
    from concourse.bass_utils import run_bass_kernel_spmd

    if "nc" not in _cache:
        _cache["nc"] = _build()
    nc = _cache["nc"]

    x = np.asarray(input, dtype=np.float32)
    adj_np = np.asarray(adj)
    W_np = np.asarray(W, dtype=np.float32)
    b_np = np.asarray(b, dtype=np.float32)
    a_np = np.asarray(a, dtype=np.float32)

    xT = np.ascontiguousarray(x.transpose(0, 2, 1))                     # [B, FIN, N]
    adjT = adj_np.transpose(0, 2, 1)                                    # [B, N(j), N(i)]
    adjm = np.where(adjT > 0, 0.0, MASK_NEG).astype(ml_dtypes.float8_e4m3fn)
    adjm = np.ascontiguousarray(adjm)
    wt = np.ascontiguousarray(W_np.T)                                   # [FIN, FOUT]
    bcol = np.ascontiguousarray(b_np.reshape(FOUT, 1))
    a12 = np.ascontiguousarray(np.stack([a_np[:FOUT, 0], a_np[FOUT:, 0]], axis=1))

    in_maps = [{"xT": xT[c], "adjm": adjm[c], "wt": wt, "bcol": bcol, "a12": a12}
               for c in range(B)]
    res = run_bass_kernel_spmd(nc, in_maps, core_ids=list(range(B)))
    out = np.stack([np.asarray(res.results[c]["outT"]).T for c in range(B)])
    return np.ascontiguousarray(out, dtype=np.float32)
